# revision 5
# baseline (speedup 1.0000x reference)
"""Bass/Trainium2 kernel for the BarlowTwins-style cross-entropy loss.

Reference (per batch b of 8):
    logits = z1[b].T @ z2[b] / T            (2048 x 2048, K=256, T=1.0)
    logp   = log_softmax(logits, axis=0)    (softmax over first axis n)
    loss   = -mean_b,m logp[m, m]

Sharding: pure data parallel over the batch axis b -> one batch element per
NeuronCore (8 cores).  Each core computes logitsT[m, n] = sum_s z2[s,m]*z1[s,n]
so the softmax reduction runs along the free axis.  Per 128-row chunk of m the
core emits:
    mx[m]  = -max_n logitsT[m, n]           (negated row max, DVE reduce)
    se[m]  = sum_n exp(logitsT[m, n] + mx[m]) (ACT exp with accumulate)
    dgblk  = logitsT[:, diag block]         (raw [128,128] block, DMA to DRAM;
                                             host gathers the diagonal)
The host combines: loss = -mean(dg + mx - log(se)).

Inputs are converted to bf16 on the host (halves DMA traffic; PE runs bf16 at
full rate with f32 PSUM accumulation; loss error vs f32 reference ~1e-4).
"""

import numpy as np
import ml_dtypes

import concourse.bass as bass
import concourse.tile as tile
from concourse import bacc, mybir
from concourse.bass_utils import run_bass_kernel_spmd

B = 8          # batch (one element per core)
S = 256        # contraction dim
N = 2048       # feature dim (n and m)
P = 128        # SBUF partitions
KC = S // P    # 2 contraction chunks
MC = N // P    # 16 row chunks of logitsT
NB = N // 512  # 4 moving-dim blocks per matmul row chunk
TEMPERATURE = 1.0

_CACHE = {}


def _build():
    if "nc" in _CACHE:
        return _CACHE["nc"]

    f32 = mybir.dt.float32
    bf16 = mybir.dt.bfloat16

    nc = bacc.Bacc("TRN2", target_bir_lowering=False, debug=False)
    z1 = nc.dram_tensor("z1", [S, N], bf16, kind="ExternalInput").ap()
    z2 = nc.dram_tensor("z2", [S, N], bf16, kind="ExternalInput").ap()
    se_d = nc.dram_tensor("se", [P, MC], f32, kind="ExternalOutput").ap()
    mx_d = nc.dram_tensor("mx", [P, MC], f32, kind="ExternalOutput").ap()
    dg_d = nc.dram_tensor("dgblk", [MC, P, P], f32, kind="ExternalOutput").ap()

    z1r = z1.rearrange("(k p) n -> k p n", p=P)
    z2r = z2.rearrange("(k p) n -> k p n", p=P)

    with tile.TileContext(nc) as tc:
        with (
            tc.tile_pool(name="const", bufs=1) as cpool,
            tc.tile_pool(name="zb", bufs=1) as zpool,
            tc.tile_pool(name="psum", bufs=2, space="PSUM") as ppool,
            tc.tile_pool(name="expout", bufs=2) as epool,
            tc.tile_pool(name="dscr", bufs=3) as dpool,
            tc.tile_pool(name="mx", bufs=4) as mpool,
        ):
            # ACT exp-table preload, overlapped with the input DMAs.
            dummy = cpool.tile([1, 1], f32, tag="dummy")
            nc.gpsimd.memset(dummy[:], 0.0)
            nc.scalar.activation(
                dummy[:], dummy[:], mybir.ActivationFunctionType.Exp, bias=0.0
            )

            se_sb = cpool.tile([P, MC], f32, tag="se_sb")

            # Input loads, split so the first matmuls can start early:
            # z2 (stationary) halves for m-chunks 0-7 first, then z1
            # (moving) fully, then the rest of z2.
            z1b = [
                zpool.tile([P, N], bf16, name=f"z1b{k}", tag=f"z1b{k}")
                for k in range(KC)
            ]
            z2b = [
                zpool.tile([P, N], bf16, name=f"z2b{k}", tag=f"z2b{k}")
                for k in range(KC)
            ]
            H = N // 2
            for k in range(KC):
                nc.sync.dma_start(z2b[k][:, 0:H], z2r[k][:, 0:H])
            for c in range(2):
                cs = slice(c * H, (c + 1) * H)
                for k in range(KC):
                    nc.sync.dma_start(z1b[k][:, cs], z1r[k][:, cs])
            for k in range(KC):
                nc.sync.dma_start(z2b[k][:, H:N], z2r[k][:, H:N])

            inv_t = 1.0 / TEMPERATURE
            for m in range(MC):
                ms = slice(m * P, (m + 1) * P)
                psum = ppool.tile([P, N], f32, tag="psum")
                for k in range(KC):
                    for nb in range(NB):
                        ns = slice(nb * 512, (nb + 1) * 512)
                        nc.tensor.matmul(
                            psum[:, ns],
                            lhsT=z2b[k][:, ms],
                            rhs=z1b[k][:, ns],
                            start=(k == 0),
                            stop=(k == KC - 1),
                        )

                # negated row max of the [P, N] chunk
                mx_t = mpool.tile([P, 1], f32, tag="mx")
                nc.vector.tensor_reduce(
                    mx_t[:],
                    psum[:],
                    axis=mybir.AxisListType.X,
                    op=mybir.AluOpType.max,
                    negate=True,
                )
                nc.sync.dma_start(mx_d[:, m : m + 1], mx_t[:])

                # raw diagonal block -> SBUF (ACT copy; DMA can't read PSUM)
                # -> DRAM; host picks out the diagonal
                dscr = dpool.tile([P, P], f32, tag="dscr")
                nc.scalar.copy(dscr[:], psum[:, ms])
                nc.sync.dma_start(dg_d[m], dscr[:])

                # exp(logitsT - rowmax), accumulated along the row
                eo = epool.tile([P, N], bf16, tag="eo")
                nc.scalar.activation(
                    eo[:],
                    psum[:],
                    mybir.ActivationFunctionType.Exp,
                    bias=mx_t[:],
                    scale=inv_t,
                    accum_out=se_sb[:, m : m + 1],
                )

            nc.sync.dma_start(se_d[:], se_sb[:])

    nc.compile()
    _CACHE["nc"] = nc
    return nc


def _run(z1, z2, **spmd_kwargs):
    """Shard over batch, run on 8 cores, return (loss, BassKernelResults)."""
    nc = _build()
    z1 = np.ascontiguousarray(z1)
    z2 = np.ascontiguousarray(z2)
    in_maps = [
        {
            "z1": np.ascontiguousarray(z1[b].astype(ml_dtypes.bfloat16)),
            "z2": np.ascontiguousarray(z2[b].astype(ml_dtypes.bfloat16)),
        }
        for b in range(B)
    ]
    res = run_bass_kernel_spmd(nc, in_maps, core_ids=list(range(B)), **spmd_kwargs)

    total = 0.0
    pidx = np.arange(P)
    for b in range(B):
        se = res.results[b]["se"].astype(np.float64)
        mx = res.results[b]["mx"].astype(np.float64)
        # dgblk[mc, p, :] holds logitsT[mc*128+p, mc*128 : (mc+1)*128];
        # the diagonal element for row m = mc*128+p sits at column p.
        dg = res.results[b]["dgblk"][:, pidx, pidx].astype(np.float64)  # [MC, P]
        # se/mx are [p, mc]; dg is [mc, p] -> align
        logZ = -mx + np.log(se)           # [p, mc]
        total += np.sum(dg.T - logZ)
    loss = -total / (B * N)
    return np.asarray(loss, dtype=np.float32), res


def kernel(z1, z2):
    loss, _ = _run(z1, z2)
    return loss
